# revision 21
# baseline (speedup 1.0000x reference)
"""DirectAU loss kernel for Trainium2, SPMD over 8 NeuronCores.

Math (see reference):
  user_e = user_table[user_id]; pos_e = item_table[pos_id]   (B=8192, D=64)
  align  = mean_i ||un_i - pn_i||^2 = 2 - (2/B) sum_i <un_i, pn_i>
  unif(x)= log( (sum_{i<j} exp(-4 + 4 <xn_i, xn_j>)) / npairs )
  out    = align + 0.5*(unif(user_e) + unif(pos_e))

Strategy (v3):
  - The two Gram computations are split across cores: cores 0-3 compute the
    user-embedding uniformity term, cores 4-7 the pos-embedding one. Both
    tables are concatenated into one [200000, 64] input, so the SPMD program
    is identical on every core and the table choice lives in the int32 gather
    indices (pos ids offset by +100000).
  - Triangular block schedule per table over 8 batch chunks of 1024: the
    per-chunk assignment a covers diag(a) at weight 1/2 (folded into the exp
    bias: exp(4s-4+ln .5)), full blocks (a,a+1..a+3), and one half of the
    distance-4 block as two 512x512 quadrants (halves swapped for a>=4, the
    swap encoded in the host-built index order). Each core takes two adjacent
    assignments {a1, a1+1}, so it gathers chunks a1..a1+5 (48 bands of 128
    rows) plus 8 bands of the OTHER table's chunk a1 for the align term.
  - Pipeline per core: 56 indirect-DMA row gathers (~1.1us each, the pacer)
    -> normalize (DVE square/reduce + Newton rsqrt; no ACT table switches)
    -> PE transpose to bf16 xnT [64, 6144] -> 144 bf16 matmuls (K=64) into
    PSUM -> ACT exp in place with accum_out row-sums into an accumulator
    tile. Emission is ordered so the diag blocks of chunk a1 start on ACT
    while later chunks are still gathering.
  - Host sums the 8x[128,64] partials and applies the closed-form log/align
    finalization (pure unshard reduction of partial sums).
"""

import math

import numpy as np

import concourse.bacc as bacc
import concourse.bass as bass
import concourse.mybir as mybir
import concourse.tile as tile
from concourse import bass_utils
from concourse.masks import make_identity

B = 8192
DIM = 64
NROWS = 100000
NCORES = 8
CHUNK = 1024
NCHUNK = 6  # gathered chunks per core (C0..C5)
MAIN_BANDS = NCHUNK * 8  # 48
AL_BANDS = 8
NBAND = MAIN_BANDS + AL_BANDS  # 56 gather bands
LN_HALF = math.log(0.5)
F32 = mybir.dt.float32
BF16 = mybir.dt.bfloat16
I32 = mybir.dt.int32

# accumulator column map: part q in {0,1}, row-tile rt in 0..7, chunk ci in
# {D, O1, O2} -> col q*24 + rt*3 + ci; align in col 48
ALIGN_COL = 48
ACC_W = 64


def _emit_rsqrt(nc, pool, x_ap, out_ap, n, tag):
    """out = 1/sqrt(x) on the vector engine (bit-hack seed + 3 Newton steps)."""
    MAGIC = 0x5F3759DF
    op = mybir.AluOpType
    ti = pool.tile([128, n], I32, tag=f"{tag}_ti", name=f"{tag}_ti")
    nc.vector.tensor_scalar(
        out=ti[:], in0=x_ap.bitcast(I32), scalar1=1, scalar2=None,
        op0=op.logical_shift_right,
    )
    yi = pool.tile([128, n], I32, tag=f"{tag}_yi", name=f"{tag}_yi")
    # MAGIC - t == (t ^ -1) + (MAGIC + 1); split: ISA can't mix bitwise+arith
    nc.vector.tensor_scalar(
        out=yi[:], in0=ti[:], scalar1=-1, scalar2=None, op0=op.bitwise_xor
    )
    nc.vector.tensor_scalar(
        out=yi[:], in0=yi[:], scalar1=MAGIC + 1, scalar2=None, op0=op.add
    )
    xh = pool.tile([128, n], F32, tag=f"{tag}_xh", name=f"{tag}_xh")
    nc.vector.tensor_scalar(
        out=xh[:], in0=x_ap, scalar1=-0.5, scalar2=None, op0=op.mult
    )
    cur = yi[:].bitcast(F32)
    for it in range(3):
        t2 = pool.tile([128, n], F32, tag=f"{tag}_t2", name=f"{tag}_t2")
        nc.vector.tensor_mul(out=t2[:], in0=cur, in1=cur)
        nc.vector.tensor_mul(out=t2[:], in0=t2[:], in1=xh[:])
        nc.vector.tensor_scalar(
            out=t2[:], in0=t2[:], scalar1=1.5, scalar2=None, op0=op.add
        )
        if it == 2:
            dst_ap = out_ap
        else:
            yt = pool.tile([128, n], F32, tag=f"{tag}_y", name=f"{tag}_y{it}")
            dst_ap = yt[:]
        nc.vector.tensor_mul(out=dst_ap, in0=cur, in1=t2[:])
        cur = dst_ap
    return cur


def _body(tc, tabs, gidx, acc):
    nc = tc.nc
    op = mybir.AluOpType
    with (
        tc.tile_pool(name="persist", bufs=1) as P,
        tc.tile_pool(name="work", bufs=2) as W,
        tc.tile_pool(name="ps", bufs=2, space="PSUM") as PS,
    ):
        ident = P.tile([128, 128], F32, tag="ident")
        make_identity(nc, ident[:])

        idx_sb = P.tile([128, NBAND], I32, tag="idx")
        nc.sync.dma_start(out=idx_sb[:], in_=gidx)

        accw = P.tile([128, ACC_W], F32, tag="accw")
        nc.gpsimd.memset(accw[:], 0.0)

        bias_o = P.tile([128, 1], F32, tag="bias_o")
        nc.gpsimd.memset(bias_o[:], -4.0)
        bias_d = P.tile([128, 1], F32, tag="bias_d")
        nc.gpsimd.memset(bias_d[:], -4.0 + LN_HALF)

        # gathered rows, [128, band, DIM] band-major slots (row c*128+p)
        gath = P.tile([128, NBAND * DIM], F32, tag="gath")
        xnT = P.tile([64, MAIN_BANDS * 128], BF16, tag="xnT")  # [64, 6144]
        nsq = P.tile([128, NBAND], F32, tag="nsq")
        rinv = P.tile([128, NBAND], F32, tag="rinv")

        def gather_band(c):
            nc.gpsimd.indirect_dma_start(
                out=gath[:, c * DIM : (c + 1) * DIM],
                out_offset=None,
                in_=tabs,
                in_offset=bass.IndirectOffsetOnAxis(
                    ap=idx_sb[:, c : c + 1], axis=0
                ),
            )

        def normalize(c0, c1, tag):
            nb = c1 - c0
            sq = W.tile([128, nb * DIM], F32, tag="sq", name=f"sq_{tag}")
            g3 = gath[:, c0 * DIM : c1 * DIM].rearrange("p (c d) -> p c d", d=DIM)
            nc.vector.tensor_tensor(out=sq[:], in0=g3, in1=g3, op=op.mult)
            nc.vector.tensor_reduce(
                out=nsq[:, c0:c1],
                in_=sq[:].rearrange("p (c d) -> p c d", d=DIM),
                axis=mybir.AxisListType.X,
                op=op.add,
            )
            _emit_rsqrt(nc, W, nsq[:, c0:c1], rinv[:, c0:c1], nb, f"nw_{tag}")
            r3 = (
                rinv[:, c0:c1]
                .rearrange("p (c o) -> p c o", o=1)
                .to_broadcast([128, nb, DIM])
            )
            nc.vector.tensor_tensor(out=g3, in0=g3, in1=r3, op=op.mult)

        def transpose_bands(c0, c1):
            for g in range(c0 // 4, c1 // 4):
                pt = PS.tile([128, 2048], F32, tag="ps", name=f"tp{g}")
                for k in range(4):
                    c = g * 4 + k
                    nc.tensor.transpose(
                        out=pt[0:64, k * 128 : (k + 1) * 128],
                        in_=gath[:, c * DIM : (c + 1) * DIM],
                        identity=ident[:],
                    )
                nc.vector.tensor_copy(
                    out=xnT[:, g * 512 : (g + 1) * 512], in_=pt[0:64, 0:512]
                )

        # col-tile j of (part q, row-tile rt):
        #   j in {0,1}: diag chunk Cq
        #   j in 2..7:  full chunks C(q+1)..C(q+3)
        #   j == 8:     quadrant into C(q+4): rt<4 -> first 512, else second
        def rhs_ap(q, rt, j):
            if j < 8:
                cs = q * 1024 + j * 512
                return xnT[:, cs : cs + 512]
            cs = (q + 4) * 1024 + (0 if rt < 4 else 512)
            return xnT[:, cs : cs + 512]

        def emit_chunk(q, rt, ci, tiles_, bias_t):
            lhs = xnT[:, q * 1024 + rt * 128 : q * 1024 + (rt + 1) * 128]
            pt = PS.tile([128, 2048], F32, tag="ps", name=f"mm{q}_{rt}_{ci}")
            w = len(tiles_) * 512
            for k, j in enumerate(tiles_):
                nc.tensor.matmul(
                    out=pt[:, k * 512 : (k + 1) * 512],
                    lhsT=lhs,
                    rhs=rhs_ap(q, rt, j),
                    start=True,
                    stop=True,
                )
            col = q * 24 + rt * 3 + ci
            nc.scalar.activation(
                out=pt[:, 0:w],
                in_=pt[:, 0:w],
                func=mybir.ActivationFunctionType.Exp,
                bias=bias_t[:],
                scale=4.0,
                accum_out=accw[:, col : col + 1],
            )

        # ---- emission in pipeline order ----
        # A pseudo-time ladder (tile_set_cur_wait) keeps the in-order engine
        # queues in chunk order: without it the scheduler interleaves ops that
        # wait on future gathers ahead of ready work (head-of-line blocking).
        # C0, C1: rows/diag of parts A and B
        for c in range(16):
            gather_band(c)
        tc.tile_set_cur_wait(0.001)
        normalize(0, 8, "c0")
        tc.tile_set_cur_wait(0.002)
        normalize(8, 16, "c1")
        transpose_bands(0, 16)
        tc.tile_set_cur_wait(0.003)
        for q in (0, 1):
            for rt in range(8):
                emit_chunk(q, rt, 0, [0, 1], bias_d)

        # remaining chunks stream in; emit their dependent matmul chunks as
        # soon as each chunk is transposed.
        for ch in range(2, 6):
            tc.tile_set_cur_wait(0.001 * (2 * ch))
            for c in range(ch * 8, (ch + 1) * 8):
                gather_band(c)
            normalize(ch * 8, (ch + 1) * 8, f"c{ch}")
            transpose_bands(ch * 8, (ch + 1) * 8)
            tc.tile_set_cur_wait(0.001 * (2 * ch + 1))
            # chunk ch serves part q as col-tile pair (2*(ch-q)), (2*(ch-q))+1
            # within O1 (j in 2..5) or O2 (j in 6,7) / quadrant (j=8)
            if ch == 2:
                for rt in range(8):
                    emit_chunk(0, rt, 1, [2, 3, 4, 5], bias_o)  # A: C1,C2 full
            if ch == 3:
                for rt in range(8):
                    emit_chunk(1, rt, 1, [2, 3, 4, 5], bias_o)  # B: C2,C3 full
            if ch == 4:
                for rt in range(8):
                    emit_chunk(0, rt, 2, [6, 7, 8], bias_o)  # A: C3 + quad C4
            if ch == 5:
                for rt in range(8):
                    emit_chunk(1, rt, 2, [6, 7, 8], bias_o)  # B: C4 + quad C5

        # align: other-table rows of chunk a1 (bands 48..55), batch order
        tc.tile_set_cur_wait(0.013)
        for c in range(MAIN_BANDS, NBAND):
            gather_band(c)
        normalize(MAIN_BANDS, NBAND, "al")
        al_sc = W.tile([128, AL_BANDS * DIM], F32, tag="alsc")
        nc.vector.tensor_mul(
            out=al_sc[:],
            in0=gath[:, 0 : AL_BANDS * DIM],
            in1=gath[:, MAIN_BANDS * DIM : NBAND * DIM],
        )
        nc.vector.tensor_reduce(
            out=accw[:, ALIGN_COL : ALIGN_COL + 1],
            in_=al_sc[:],
            axis=mybir.AxisListType.X,
            op=op.add,
        )

        nc.sync.dma_start(out=acc, in_=accw[:])


def _build():
    nc = bacc.Bacc(
        "TRN2",
        target_bir_lowering=False,
        debug=False,
        enable_asserts=False,
        num_devices=NCORES,
    )
    tabs = nc.dram_tensor("tabs", [2 * NROWS, DIM], F32, kind="ExternalInput").ap()
    gidx = nc.dram_tensor("gidx", [128, NBAND], I32, kind="ExternalInput").ap()
    acc = nc.dram_tensor("acc", [128, ACC_W], F32, kind="ExternalOutput").ap()
    with tile.TileContext(nc) as tc:
        _body(tc, tabs, gidx, acc)
    nc.compile()
    return nc


_PROG = None


def _get_prog():
    global _PROG
    if _PROG is None:
        _PROG = _build()
    return _PROG


def _core_params(m):
    """core m -> (table t, first assignment a1)."""
    t = 0 if m < 4 else 1
    j = m % 4
    a1 = 2 * j + t  # u-cores: 0,2,4,6; p-cores: 1,3,5,7
    return t, a1


def _core_gidx(uid, pid, m):
    """[128, NBAND] int32 gather indices for core m (into the concat table)."""
    t, a1 = _core_params(m)
    main_ids = [uid, pid][t]
    other_ids = [uid, pid][1 - t]
    ch = main_ids.reshape(NCORES, CHUNK)
    och = other_ids.reshape(NCORES, CHUNK)

    def h(a):  # quadrant half order for assignment a
        return 0 if a < 4 else 1

    segs = []
    for i in range(NCHUNK):
        cids = ch[(a1 + i) % NCORES].astype(np.int64) + t * NROWS
        if i == 4 and h(a1) == 1:
            cids = np.concatenate([cids[512:], cids[:512]])
        if i == 5 and h((a1 + 1) % NCORES) == 1:
            cids = np.concatenate([cids[512:], cids[:512]])
        segs.append(cids)
    # align: other table's chunk a1, batch order
    segs.append(och[a1].astype(np.int64) + (1 - t) * NROWS)
    slots = np.concatenate(segs).astype(np.int32)
    assert slots.shape == (NBAND * 128,)
    return np.ascontiguousarray(slots.reshape(NBAND, 128).T)


def _make_in_maps(user_id, pos_id, user_table, item_table):
    tabs = np.ascontiguousarray(
        np.concatenate(
            [
                np.asarray(user_table, dtype=np.float32),
                np.asarray(item_table, dtype=np.float32),
            ],
            axis=0,
        )
    )
    uid = np.asarray(user_id).astype(np.int64)
    pid = np.asarray(pos_id).astype(np.int64)
    return [
        {"tabs": tabs, "gidx": _core_gidx(uid, pid, m)} for m in range(NCORES)
    ]


def _finalize(accs):
    """accs: list of [128, ACC_W] per core -> scalar loss."""
    a = np.stack([np.asarray(x, dtype=np.float64) for x in accs])  # [8,128,64]
    s_u = a[0:4, :, 0:48].sum()
    s_p = a[4:8, :, 0:48].sum()
    s_al = a[:, :, ALIGN_COL].sum()
    npairs = B * (B - 1) // 2
    pair_u = s_u - B / 2.0
    pair_p = s_p - B / 2.0
    unif = 0.5 * (np.log(pair_u / npairs) + np.log(pair_p / npairs))
    align = 2.0 - (2.0 / B) * s_al
    return np.asarray(align + unif, dtype=np.float32)


def _run(in_maps, trace=False, **kw):
    nc = _get_prog()
    return bass_utils.run_bass_kernel_spmd(
        nc, in_maps, core_ids=list(range(NCORES)), trace=trace, **kw
    )


def kernel(user_id, pos_id, neg_id=None, user_table=None, item_table=None):
    in_maps = _make_in_maps(user_id, pos_id, user_table, item_table)
    res = _run(in_maps, trace=False)
    return _finalize([res.results[m]["acc"] for m in range(NCORES)])


def _install_profile_hook():
    """The image's antenv lacks axon_hooks; shim it so trace=True can reach
    the NTFF profiler in libaxon_pjrt.so (same mechanism trn_boot uses)."""
    import sys
    import types

    if "antenv.axon_hooks" in sys.modules:
        return
    import antenv
    from trn_agent_boot.trn_boot import _ntff_profile_via_ctypes

    mod = types.ModuleType("antenv.axon_hooks")
    holder = [None]
    mod.set_axon_ntff_profile_hook = lambda h: holder.__setitem__(0, h)
    mod.get_axon_ntff_profile_hook = lambda: holder[0]
    sys.modules["antenv.axon_hooks"] = mod
    antenv.axon_hooks = mod
    mod.set_axon_ntff_profile_hook(
        _ntff_profile_via_ctypes("/opt/axon/libaxon_pjrt.so")
    )
    # no bucket filesystem in this container
    bass_utils.upload_artifacts = lambda tmpdir: ""


def run_profiled(user_id, pos_id, neg_id=None, user_table=None, item_table=None, **kw):
    _install_profile_hook()
    in_maps = _make_in_maps(user_id, pos_id, user_table, item_table)
    res = _run(in_maps, trace=True, **kw)
    out = _finalize([res.results[m]["acc"] for m in range(NCORES)])
    return out, res


# revision 23
# speedup vs baseline: 1.0072x; 1.0072x over previous
"""DirectAU loss kernel for Trainium2, SPMD over 8 NeuronCores.

Math (see reference):
  user_e = user_table[user_id]; pos_e = item_table[pos_id]   (B=8192, D=64)
  align  = mean_i ||un_i - pn_i||^2 = 2 - (2/B) sum_i <un_i, pn_i>
  unif(x)= log( (sum_{i<j} exp(-4 + 4 <xn_i, xn_j>)) / npairs )
  out    = align + 0.5*(unif(user_e) + unif(pos_e))

Strategy (v3):
  - The two Gram computations are split across cores: cores 0-3 compute the
    user-embedding uniformity term, cores 4-7 the pos-embedding one. Both
    tables are concatenated into one [200000, 64] input, so the SPMD program
    is identical on every core and the table choice lives in the int32 gather
    indices (pos ids offset by +100000).
  - Triangular block schedule per table over 8 batch chunks of 1024: the
    per-chunk assignment a covers diag(a) at weight 1/2 (folded into the exp
    bias: exp(4s-4+ln .5)), full blocks (a,a+1..a+3), and one half of the
    distance-4 block as two 512x512 quadrants (halves swapped for a>=4, the
    swap encoded in the host-built index order). Each core takes two adjacent
    assignments {a1, a1+1}, so it gathers chunks a1..a1+5 (48 bands of 128
    rows) plus 8 bands of the OTHER table's chunk a1 for the align term.
  - Pipeline per core: 56 indirect-DMA row gathers (~1.1us each, the pacer)
    -> normalize (DVE square/reduce + Newton rsqrt; no ACT table switches)
    -> PE transpose to bf16 xnT [64, 6144] -> 144 bf16 matmuls (K=64) into
    PSUM -> ACT exp in place with accum_out row-sums into an accumulator
    tile. Emission is ordered so the diag blocks of chunk a1 start on ACT
    while later chunks are still gathering.
  - Host sums the 8x[128,64] partials and applies the closed-form log/align
    finalization (pure unshard reduction of partial sums).
"""

import math

import numpy as np

import concourse.bacc as bacc
import concourse.bass as bass
import concourse.mybir as mybir
import concourse.tile as tile
from concourse import bass_utils
from concourse.masks import make_identity

B = 8192
DIM = 64
NROWS = 100000
NCORES = 8
CHUNK = 1024
NCHUNK = 6  # gathered chunks per core (C0..C5)
MAIN_BANDS = NCHUNK * 8  # 48
AL_BANDS = 8
NBAND = MAIN_BANDS + AL_BANDS  # 56 gather bands
LN_HALF = math.log(0.5)
F32 = mybir.dt.float32
BF16 = mybir.dt.bfloat16
I32 = mybir.dt.int32

# accumulator column map: part q in {0,1}, row-tile rt in 0..7, chunk ci in
# {D, O1, O2} -> col q*24 + rt*3 + ci; align in col 48
ALIGN_COL = 48
ACC_W = 64


def _emit_rsqrt(nc, pool, x_ap, out_ap, n, tag):
    """out = 1/sqrt(x) on the vector engine (bit-hack seed + 3 Newton steps)."""
    MAGIC = 0x5F3759DF
    op = mybir.AluOpType
    ti = pool.tile([128, n], I32, tag=f"{tag}_ti", name=f"{tag}_ti")
    nc.vector.tensor_scalar(
        out=ti[:], in0=x_ap.bitcast(I32), scalar1=1, scalar2=None,
        op0=op.logical_shift_right,
    )
    yi = pool.tile([128, n], I32, tag=f"{tag}_yi", name=f"{tag}_yi")
    # MAGIC - t == (t ^ -1) + (MAGIC + 1); split: ISA can't mix bitwise+arith
    nc.vector.tensor_scalar(
        out=yi[:], in0=ti[:], scalar1=-1, scalar2=None, op0=op.bitwise_xor
    )
    nc.vector.tensor_scalar(
        out=yi[:], in0=yi[:], scalar1=MAGIC + 1, scalar2=None, op0=op.add
    )
    xh = pool.tile([128, n], F32, tag=f"{tag}_xh", name=f"{tag}_xh")
    nc.vector.tensor_scalar(
        out=xh[:], in0=x_ap, scalar1=-0.5, scalar2=None, op0=op.mult
    )
    cur = yi[:].bitcast(F32)
    for it in range(3):
        t2 = pool.tile([128, n], F32, tag=f"{tag}_t2", name=f"{tag}_t2")
        nc.vector.tensor_mul(out=t2[:], in0=cur, in1=cur)
        nc.vector.tensor_mul(out=t2[:], in0=t2[:], in1=xh[:])
        nc.vector.tensor_scalar(
            out=t2[:], in0=t2[:], scalar1=1.5, scalar2=None, op0=op.add
        )
        if it == 2:
            dst_ap = out_ap
        else:
            yt = pool.tile([128, n], F32, tag=f"{tag}_y", name=f"{tag}_y{it}")
            dst_ap = yt[:]
        nc.vector.tensor_mul(out=dst_ap, in0=cur, in1=t2[:])
        cur = dst_ap
    return cur


def _body(tc, tabs, gidx, acc):
    nc = tc.nc
    op = mybir.AluOpType
    with (
        tc.tile_pool(name="persist", bufs=1) as P,
        tc.tile_pool(name="work", bufs=2) as W,
        tc.tile_pool(name="ps", bufs=2, space="PSUM") as PS,
    ):
        ident = P.tile([128, 128], F32, tag="ident")
        make_identity(nc, ident[:])

        idx_sb = P.tile([128, NBAND], I32, tag="idx")
        nc.sync.dma_start(out=idx_sb[:], in_=gidx)

        accw = P.tile([128, ACC_W], F32, tag="accw")
        nc.gpsimd.memset(accw[:], 0.0)

        bias_o = P.tile([128, 1], F32, tag="bias_o")
        nc.gpsimd.memset(bias_o[:], -4.0)
        bias_d = P.tile([128, 1], F32, tag="bias_d")
        nc.gpsimd.memset(bias_d[:], -4.0 + LN_HALF)

        # gathered rows, [128, band, DIM] band-major slots (row c*128+p)
        gath = P.tile([128, NBAND * DIM], F32, tag="gath")
        xnT = P.tile([64, MAIN_BANDS * 128], BF16, tag="xnT")  # [64, 6144]
        nsq = P.tile([128, NBAND], F32, tag="nsq")
        rinv = P.tile([128, NBAND], F32, tag="rinv")

        def gather_band(c):
            nc.gpsimd.indirect_dma_start(
                out=gath[:, c * DIM : (c + 1) * DIM],
                out_offset=None,
                in_=tabs,
                in_offset=bass.IndirectOffsetOnAxis(
                    ap=idx_sb[:, c : c + 1], axis=0
                ),
            )

        def normalize(c0, c1, tag):
            nb = c1 - c0
            sq = W.tile([128, nb * DIM], F32, tag="sq", name=f"sq_{tag}")
            g3 = gath[:, c0 * DIM : c1 * DIM].rearrange("p (c d) -> p c d", d=DIM)
            nc.vector.tensor_tensor(out=sq[:], in0=g3, in1=g3, op=op.mult)
            nc.vector.tensor_reduce(
                out=nsq[:, c0:c1],
                in_=sq[:].rearrange("p (c d) -> p c d", d=DIM),
                axis=mybir.AxisListType.X,
                op=op.add,
            )
            _emit_rsqrt(nc, W, nsq[:, c0:c1], rinv[:, c0:c1], nb, f"nw_{tag}")
            r3 = (
                rinv[:, c0:c1]
                .rearrange("p (c o) -> p c o", o=1)
                .to_broadcast([128, nb, DIM])
            )
            nc.vector.tensor_tensor(out=g3, in0=g3, in1=r3, op=op.mult)

        def transpose_bands(c0, c1):
            for g in range(c0 // 4, c1 // 4):
                pt = PS.tile([128, 2048], F32, tag="ps", name=f"tp{g}")
                for k in range(4):
                    c = g * 4 + k
                    nc.tensor.transpose(
                        out=pt[0:64, k * 128 : (k + 1) * 128],
                        in_=gath[:, c * DIM : (c + 1) * DIM],
                        identity=ident[:],
                    )
                nc.vector.tensor_copy(
                    out=xnT[:, g * 512 : (g + 1) * 512], in_=pt[0:64, 0:512]
                )

        # col-tile j of (part q, row-tile rt):
        #   j in {0,1}: diag chunk Cq
        #   j in 2..7:  full chunks C(q+1)..C(q+3)
        #   j == 8:     quadrant into C(q+4): rt<4 -> first 512, else second
        def rhs_ap(q, rt, j):
            if j < 8:
                cs = q * 1024 + j * 512
                return xnT[:, cs : cs + 512]
            cs = (q + 4) * 1024 + (0 if rt < 4 else 512)
            return xnT[:, cs : cs + 512]

        def emit_chunk(q, rt, ci, tiles_, bias_t):
            lhs = xnT[:, q * 1024 + rt * 128 : q * 1024 + (rt + 1) * 128]
            pt = PS.tile([128, 2048], F32, tag="ps", name=f"mm{q}_{rt}_{ci}")
            w = len(tiles_) * 512
            for k, j in enumerate(tiles_):
                nc.tensor.matmul(
                    out=pt[:, k * 512 : (k + 1) * 512],
                    lhsT=lhs,
                    rhs=rhs_ap(q, rt, j),
                    start=True,
                    stop=True,
                )
            col = q * 24 + rt * 3 + ci
            nc.scalar.activation(
                out=pt[:, 0:w],
                in_=pt[:, 0:w],
                func=mybir.ActivationFunctionType.Exp,
                bias=bias_t[:],
                scale=4.0,
                accum_out=accw[:, col : col + 1],
            )

        # ---- emission: software-pipelined stages ----
        # The per-engine queues are in-order, so each transpose group is
        # emitted right before the matmul stage that first needs it, and every
        # gather/normalize stage is emitted one stage ahead of its consumers.
        # MM stages (each 8 chunks): D(A), D(B), O1(A), O1(B), O2(A), O2(B)
        mm_stages = [
            (0, 0, [0, 1], bias_d),
            (1, 0, [0, 1], bias_d),
            (0, 1, [2, 3, 4, 5], bias_o),
            (1, 1, [2, 3, 4, 5], bias_o),
            (0, 2, [6, 7, 8], bias_o),
            (1, 2, [6, 7, 8], bias_o),
        ]

        def gn(ch, tag):  # gather + normalize chunk ch
            for c in range(ch * 8, (ch + 1) * 8):
                gather_band(c)
            normalize(ch * 8, (ch + 1) * 8, tag)

        gn(0, "c0")
        transpose_bands(0, 8)  # T(C0)
        gn(1, "c1")
        for rt in range(8):  # D(A): needs C0 only
            emit_chunk(0, rt, 0, [0, 1], bias_d)
        transpose_bands(8, 16)  # T(C1)
        gn(2, "c2")
        for rt in range(8):  # D(B): needs C1
            emit_chunk(1, rt, 0, [0, 1], bias_d)
        transpose_bands(16, 24)  # T(C2)
        gn(3, "c3")
        for rt in range(8):  # O1(A): needs C1, C2
            emit_chunk(0, rt, 1, [2, 3, 4, 5], bias_o)
        transpose_bands(24, 32)  # T(C3)
        gn(4, "c4")
        for rt in range(8):  # O1(B): needs C2, C3
            emit_chunk(1, rt, 1, [2, 3, 4, 5], bias_o)
        transpose_bands(32, 40)  # T(C4)
        gn(5, "c5")
        for rt in range(8):  # O2(A): needs C3 + quad C4
            emit_chunk(0, rt, 2, [6, 7, 8], bias_o)
        transpose_bands(40, 48)  # T(C5)
        # align gathers last (they gate nothing but the tiny align dot)
        for c in range(MAIN_BANDS, NBAND):
            gather_band(c)
        for rt in range(8):  # O2(B): needs C4 + quad C5
            emit_chunk(1, rt, 2, [6, 7, 8], bias_o)
        normalize(MAIN_BANDS, NBAND, "al")
        al_sc = W.tile([128, AL_BANDS * DIM], F32, tag="alsc")
        nc.vector.tensor_mul(
            out=al_sc[:],
            in0=gath[:, 0 : AL_BANDS * DIM],
            in1=gath[:, MAIN_BANDS * DIM : NBAND * DIM],
        )
        nc.vector.tensor_reduce(
            out=accw[:, ALIGN_COL : ALIGN_COL + 1],
            in_=al_sc[:],
            axis=mybir.AxisListType.X,
            op=op.add,
        )

        nc.sync.dma_start(out=acc, in_=accw[:])


def _build():
    nc = bacc.Bacc(
        "TRN2",
        target_bir_lowering=False,
        debug=False,
        enable_asserts=False,
        num_devices=NCORES,
    )
    tabs = nc.dram_tensor("tabs", [2 * NROWS, DIM], F32, kind="ExternalInput").ap()
    gidx = nc.dram_tensor("gidx", [128, NBAND], I32, kind="ExternalInput").ap()
    acc = nc.dram_tensor("acc", [128, ACC_W], F32, kind="ExternalOutput").ap()
    with tile.TileContext(nc) as tc:
        _body(tc, tabs, gidx, acc)
    nc.compile()
    return nc


_PROG = None


def _get_prog():
    global _PROG
    if _PROG is None:
        _PROG = _build()
    return _PROG


def _core_params(m):
    """core m -> (table t, first assignment a1)."""
    t = 0 if m < 4 else 1
    j = m % 4
    a1 = 2 * j + t  # u-cores: 0,2,4,6; p-cores: 1,3,5,7
    return t, a1


def _core_gidx(uid, pid, m):
    """[128, NBAND] int32 gather indices for core m (into the concat table)."""
    t, a1 = _core_params(m)
    main_ids = [uid, pid][t]
    other_ids = [uid, pid][1 - t]
    ch = main_ids.reshape(NCORES, CHUNK)
    och = other_ids.reshape(NCORES, CHUNK)

    def h(a):  # quadrant half order for assignment a
        return 0 if a < 4 else 1

    segs = []
    for i in range(NCHUNK):
        cids = ch[(a1 + i) % NCORES].astype(np.int64) + t * NROWS
        if i == 4 and h(a1) == 1:
            cids = np.concatenate([cids[512:], cids[:512]])
        if i == 5 and h((a1 + 1) % NCORES) == 1:
            cids = np.concatenate([cids[512:], cids[:512]])
        segs.append(cids)
    # align: other table's chunk a1, batch order
    segs.append(och[a1].astype(np.int64) + (1 - t) * NROWS)
    slots = np.concatenate(segs).astype(np.int32)
    assert slots.shape == (NBAND * 128,)
    return np.ascontiguousarray(slots.reshape(NBAND, 128).T)


def _make_in_maps(user_id, pos_id, user_table, item_table):
    tabs = np.ascontiguousarray(
        np.concatenate(
            [
                np.asarray(user_table, dtype=np.float32),
                np.asarray(item_table, dtype=np.float32),
            ],
            axis=0,
        )
    )
    uid = np.asarray(user_id).astype(np.int64)
    pid = np.asarray(pos_id).astype(np.int64)
    return [
        {"tabs": tabs, "gidx": _core_gidx(uid, pid, m)} for m in range(NCORES)
    ]


def _finalize(accs):
    """accs: list of [128, ACC_W] per core -> scalar loss."""
    a = np.stack([np.asarray(x, dtype=np.float64) for x in accs])  # [8,128,64]
    s_u = a[0:4, :, 0:48].sum()
    s_p = a[4:8, :, 0:48].sum()
    s_al = a[:, :, ALIGN_COL].sum()
    npairs = B * (B - 1) // 2
    pair_u = s_u - B / 2.0
    pair_p = s_p - B / 2.0
    unif = 0.5 * (np.log(pair_u / npairs) + np.log(pair_p / npairs))
    align = 2.0 - (2.0 / B) * s_al
    return np.asarray(align + unif, dtype=np.float32)


def _run(in_maps, trace=False, **kw):
    nc = _get_prog()
    return bass_utils.run_bass_kernel_spmd(
        nc, in_maps, core_ids=list(range(NCORES)), trace=trace, **kw
    )


def kernel(user_id, pos_id, neg_id=None, user_table=None, item_table=None):
    in_maps = _make_in_maps(user_id, pos_id, user_table, item_table)
    res = _run(in_maps, trace=False)
    return _finalize([res.results[m]["acc"] for m in range(NCORES)])


def _install_profile_hook():
    """The image's antenv lacks axon_hooks; shim it so trace=True can reach
    the NTFF profiler in libaxon_pjrt.so (same mechanism trn_boot uses)."""
    import sys
    import types

    if "antenv.axon_hooks" in sys.modules:
        return
    import antenv
    from trn_agent_boot.trn_boot import _ntff_profile_via_ctypes

    mod = types.ModuleType("antenv.axon_hooks")
    holder = [None]
    mod.set_axon_ntff_profile_hook = lambda h: holder.__setitem__(0, h)
    mod.get_axon_ntff_profile_hook = lambda: holder[0]
    sys.modules["antenv.axon_hooks"] = mod
    antenv.axon_hooks = mod
    mod.set_axon_ntff_profile_hook(
        _ntff_profile_via_ctypes("/opt/axon/libaxon_pjrt.so")
    )
    # no bucket filesystem in this container
    bass_utils.upload_artifacts = lambda tmpdir: ""


def run_profiled(user_id, pos_id, neg_id=None, user_table=None, item_table=None, **kw):
    _install_profile_hook()
    in_maps = _make_in_maps(user_id, pos_id, user_table, item_table)
    res = _run(in_maps, trace=True, **kw)
    out = _finalize([res.results[m]["acc"] for m in range(NCORES)])
    return out, res
